# revision 39
# baseline (speedup 1.0000x reference)
"""Trainium2 Bass kernel for a 4-layer GCN (nn_GCNNet).

Strategy (8 NeuronCores, SPMD single NEFF):
  - Core c owns the contiguous node range [c*6250, (c+1)*6250) and all edges
    whose dst falls in that range (edge sharding by destination).
  - Node features h live transposed in SBUF as hT [128 d, 6250 nodes] f32.
  - Per GCN layer: every core gathers h[src] rows for its edges from a
    replicated DRAM copy of h (dma_gather, 256B bf16 rows), aggregates them
    into m^T per 128-dst-node block with one-hot matmuls accumulating in
    PSUM (the fp8 one-hot carries the symmetric-norm coefficient per edge
    and is precomputed host-side, streamed from DRAM), applies the layer
    weight as a [128x128] @ [128x512] matmul, relu+bias on the scalar
    engine, residual-adds into hT, and publishes its updated node shard via
    Shared-output AllGather so every core has the full h for the next layer.
  - dma_gather indices are int16, so the gather source is addressed as two
    ~25k-row halves (A = first 24 blocks per core, B = rest), published by
    two separate AllGathers. Gather descriptor generation runs on Q7 core
    pairs selected by the SWDGE queue number, so gathers are striped
    round-robin over queues 0-3 (and split in half) to overlap the
    per-index descriptor-generation cost 4 ways.
  - Each layer runs in two phases: phase 1 aggregates own+A-sourced edge
    tiles for every block (partials parked in SBUF); phase 2 lags LAG
    chunks behind, adds the B-sourced tiles, applies W, the residual, and
    the writeback. The lag keeps compute flowing while the previous
    layer's AllGather of the B half is still in flight.
  - Edges within each (block, region) segment are sorted by gather index
    (ascending HBM addresses per descriptor batch) and edges sharing a
    source collapse into one gathered slot with a multi-hot column.
  - MLP readout (128->64->32->128) runs on the transposed features, then
    tiles are transposed back via the PE and DMA'd out.

Host-side work is limited to graph preprocessing: sharding/sorting edges,
padding, building index streams and fp8 one-hot slabs, degree counts and
the norm coefficients isq_src[src]*isq_dst[dst] (pure functions of the
integer edge lists), plus the constant sinusoidal position table. All
tensor math (embedding lookup, aggregation, matmuls, activations,
residuals, readout) runs on device.
"""

import os
import sys

sys.path.insert(0, "/opt/trn_rl_repo")

import math

import numpy as np

import concourse.bacc as bacc
import concourse.bass as bass
import concourse.mybir as mybir
import concourse.tile as tile
from concourse.bass_utils import run_bass_kernel_spmd

# Problem constants (hardcoded per contest rules).
N_GRAPHS = 25
NODES_PER = 2000
N = N_GRAPHS * NODES_PER          # 50000
E = 800000
D = 128
VOCAB = 30
NLAYERS = 4
NCORES = 8
NPC = N // NCORES                 # 6250 nodes per core
HBLK = 24                         # blocks per AG1 prefix ("A" half)
AROWS = HBLK * 128                # 3072 rows per core in the A half
BROWS = NPC - AROWS               # 3178 rows per core in the B half
NB = (NPC + 127) // 128           # 49 dst blocks / node tiles per core
LAST_ROWS = NPC - 128 * (NB - 1)  # 106 valid rows in the last tile
NSLOT = NB * 128                  # 6272 padded node slots
CHUNK_NB = 4                      # dst blocks per gather chunk (= W-matmul group)
# fp8 pair tables: h rows stored as [pair, 256] fp8 (2 nodes / 256B row),
# values pre-scaled by HSCALE (fp8e4m3 subnormal floor) with 1/HSCALE folded
# into the layer weights.
APAIRS = AROWS // 2               # 1536 pairs per core in the A half
BPAIRS = BROWS // 2               # 1589 pairs per core in the B half
PPAIRS = NSLOT // 2               # 3136 pair rows in the local table
NAP = NCORES * APAIRS             # 12288 pair rows in hgA
NBP = NCORES * BPAIRS             # 12712 pair rows in hgB
HSCALE = 64.0

F32 = mybir.dt.float32
BF16 = mybir.dt.bfloat16
F8 = mybir.dt.float8e4
I16 = mybir.dt.int16

_cache = {}


def _pos_table():
    pos = (np.arange(NODES_PER, dtype=np.float64) + 1.0)[:, None]
    div = np.exp(np.arange(0, D, 2, dtype=np.float64) * (-math.log(10000.0) / D))
    ang = pos * div
    tab = np.stack([np.sin(ang), np.cos(ang)], axis=-1).reshape(NODES_PER, D)
    return tab.astype(np.float32)


def _wrap16(stream):
    """int16 index stream -> [128, len/16] SBUF layout (16-partition wrap,
    replicated to all 8 gpsimd cores)."""
    v = stream.reshape(-1, 16).T  # [16, cols]
    return np.tile(v, (8, 1)).astype(np.int16)


def _balance_partition(deg_vec):
    """Assign nodes to 8 cores (6250 each), balancing total in-degree.
    Returns old_of_new: new label -> old node id."""
    order = np.argsort(-deg_vec, kind="stable")
    loads = np.zeros(NCORES)
    counts = np.zeros(NCORES, np.int64)
    assign = np.empty(N, np.int64)
    for v in order:
        c = int(np.argmin(np.where(counts < NPC, loads, np.inf)))
        assign[v] = c
        loads[c] += deg_vec[v]
        counts[c] += 1
    old_of = np.empty(N, np.int64)
    pos = np.zeros(NCORES, np.int64)
    # blocks are packed later; here order within a core is provisional
    for v in np.arange(N):
        c = assign[v]
        old_of[c * NPC + pos[c]] = v
        pos[c] += 1
    return assign, old_of


def _caps2(nfat=16):
    c = np.tile(np.array([256, 1792], np.int64), (NB, 1))
    c[:nfat] = (384, 2048)
    return c


def _caps3(nfat):
    c = np.tile(np.array([256, 896, 896], np.int64), (NB, 1))
    c[:nfat] = (384, 1024, 1024)
    return c


def _pack_blocks(nodes_old, wmat, caps, init_members=None):
    if caps.ndim == 1:
        caps = np.tile(caps, (NB, 1))
    """Pack one core's 6250 nodes into 49 blocks (last=106 nodes) under
    per-block edge quotas; lowest-index-first so fill patterns align across
    cores (tile counts are cross-core maxes)."""
    order = np.argsort(-wmat.sum(1), kind="stable")
    ncaps = caps.shape[-1]
    if init_members is not None:
        members = [list(m) for m in init_members]
        node_w = {int(nodes_old[i]): wmat[i] for i in range(len(nodes_old))}
        loads = np.zeros((NB, ncaps), np.int64)
        for b in range(NB):
            for v in members[b]:
                loads[b] += node_w[v]
        return _refine(members, node_w, loads, caps)
    loads = np.zeros((NB, ncaps), np.int64)
    counts = np.zeros(NB, np.int64)
    block_cap = np.full(NB, 128, np.int64)
    block_cap[NB - 1] = LAST_ROWS
    members = [[] for _ in range(NB)]
    for i in order:
        v = nodes_old[i]
        wv = wmat[i]
        fits = (counts[:-1] < block_cap[:-1]) & np.all(
            loads[:-1] + wv[None, :] <= caps[:-1], axis=1
        )
        if fits.any():
            b = int(np.argmax(fits))
        elif counts[NB - 1] < block_cap[NB - 1]:
            b = NB - 1
        else:
            over = ((loads[:-1] + wv[None, :]) / caps[:-1]).max(1)
            over[counts[:-1] >= block_cap[:-1]] = np.inf
            b = NB - 2 - int(np.argmin(over[::-1]))
        members[b].append(v)
        loads[b] += wv
        counts[b] += 1
    assert all(len(members[b]) == block_cap[b] for b in range(NB))
    node_w = {int(nodes_old[i]): wmat[i] for i in range(len(nodes_old))}
    loads = np.zeros((NB, wmat.shape[1]), np.int64)
    for b in range(NB):
        for v in members[b]:
            loads[b] += node_w[v]
    return _refine(members, node_w, loads, caps)


def _refine(members, node_w, loads, caps):
    for _ in range(4000):
        over = (loads[:-1] - caps[:-1]).max(1)
        b = int(np.argmax(over))
        if over[b] <= 0:
            break
        d = int(np.argmax(loads[b] - caps[b]))
        # candidate donors: big-w[d] nodes of b; receivers: slackiest block
        done = False
        for b2 in np.argsort(-(caps[:-1, d] - loads[:-1, d]))[:6]:
            if b2 == b:
                continue
            mw = [node_w[v][d] for v in members[b]]
            for ui in np.argsort(mw)[::-1][:8]:
                u = members[b][int(ui)]
                wu = node_w[u]
                for vi, v in enumerate(members[b2][:64]):
                    wv = node_w[v]
                    delta = wu - wv
                    if delta[d] <= 0:
                        continue
                    nb = loads[b] - delta
                    nb2 = loads[b2] + delta
                    if (nb2 <= caps[b2]).all() and (nb - caps[b]).max() < over[b]:
                        members[b][int(ui)] = v
                        members[b2][vi] = u
                        loads[b] = nb
                        loads[b2] = nb2
                        done = True
                        break
                if done:
                    break
            if done:
                break
        if not done:
            break
    return members


def _label_from_blocks(assign, blocks_per_core):
    old_of = np.empty(N, np.int64)
    p = 0
    for c in range(NCORES):
        for b in range(NB):
            for v in blocks_per_core[c][b]:
                old_of[p] = v
                p += 1
    newid = np.empty(N, np.int64)
    newid[old_of] = np.arange(N)
    return old_of, newid


def _preprocess(labels, src, dst, perms):
    """Relabel/shard/sort/pad edges; build per-core device input arrays."""
    src = np.asarray(src).astype(np.int64)
    dst = np.asarray(dst).astype(np.int64)
    labels = np.asarray(labels).astype(np.int64)
    perms = np.asarray(perms).astype(np.int64)

    deg_out = np.bincount(src, minlength=N)
    deg_in = np.bincount(dst, minlength=N)
    isq_src = (np.maximum(deg_out, 1) ** -0.5).astype(np.float32)
    isq_dst = (np.maximum(deg_in, 1) ** -0.5).astype(np.float32)
    se_all = (isq_src[src] * isq_dst[dst]).astype(np.float32)

    # step 1: balanced core assignment (by in-degree)
    assign, _ = _balance_partition(deg_in.astype(np.float64))
    src_core = assign[src]
    own_edge = src_core == assign[dst]
    d_own = np.bincount(dst[own_edge], minlength=N)
    d_no = np.bincount(dst[~own_edge], minlength=N)

    # step 2 round 1: pack by (own, nonown) to get provisional labels
    blocks1 = []
    for c in range(NCORES):
        nodes_c = np.where(assign == c)[0]
        w = np.stack([d_own[nodes_c], d_no[nodes_c]], 1)
        blocks1.append(_pack_blocks(nodes_c, w, _caps2()))
    old_of, newid = _label_from_blocks(assign, blocks1)

    # step 2 round 2: A = src in first HBLK blocks of its core; repack with
    # (own, A, B) quotas using round-1 membership as the estimate
    in_a = (newid[src] % NPC) < AROWS
    d_a = np.bincount(dst[(~own_edge) & in_a], minlength=N)
    d_b = np.bincount(dst[(~own_edge) & ~in_a], minlength=N)
    loads_ab = np.zeros((NCORES, 2), np.int64)
    for c in range(NCORES):
        m = assign == c
        loads_ab[c] = (d_a[m].sum(), d_b[m].sum())
    nfat = int(min(48, np.ceil((loads_ab.max() - 48 * 896) / 128) + 6))
    nfat = max(nfat, 0)
    blocks2 = []
    for c in range(NCORES):
        nodes_c = np.where(assign == c)[0]
        w = np.stack([d_own[nodes_c], d_a[nodes_c], d_b[nodes_c]], 1)
        blocks2.append(_pack_blocks(nodes_c, w, _caps3(nfat)))
    old_of, newid = _label_from_blocks(assign, blocks2)

    # round 3: one more iteration with refreshed A/B membership
    in_a = (newid[src] % NPC) < AROWS
    d_a = np.bincount(dst[(~own_edge) & in_a], minlength=N)
    d_b = np.bincount(dst[(~own_edge) & ~in_a], minlength=N)
    blocks3 = []
    for c in range(NCORES):
        nodes_c = np.where(assign == c)[0]
        w = np.stack([d_own[nodes_c], d_a[nodes_c], d_b[nodes_c]], 1)
        blocks3.append(
            _pack_blocks(nodes_c, w, _caps3(nfat), init_members=blocks2[c])
        )
    old_of, newid = _label_from_blocks(assign, blocks3)

    # round 4: refine once more against refreshed membership
    in_a = (newid[src] % NPC) < AROWS
    d_a = np.bincount(dst[(~own_edge) & in_a], minlength=N)
    d_b = np.bincount(dst[(~own_edge) & ~in_a], minlength=N)
    blocks4 = []
    for c in range(NCORES):
        nodes_c = np.where(assign == c)[0]
        w = np.stack([d_own[nodes_c], d_a[nodes_c], d_b[nodes_c]], 1)
        blocks4.append(
            _pack_blocks(nodes_c, w, _caps3(nfat), init_members=blocks3[c])
        )
    old_of, newid = _label_from_blocks(assign, blocks4)

    src_n = newid[src]
    dst_n = newid[dst]

    # step 3: edge grouping on FINAL labels
    core = dst_n // NPC
    dstloc = dst_n % NPC
    blk = dstloc >> 7
    dl = (dstloc & 127).astype(np.float32)
    src_core_n = src_n // NPC
    src_loc = src_n % NPC
    own = src_core_n == core
    in_a = src_loc < AROWS
    region = np.where(own, 0, np.where(in_a, 1, 2))
    # gather index per edge by region (PAIR index into the fp8 tables)
    parity = (src_loc & 1).astype(np.int64)
    g_idx = np.where(
        own,
        src_loc >> 1,
        np.where(
            in_a,
            src_core_n * APAIRS + (src_loc >> 1),
            src_core_n * BPAIRS + ((src_loc - AROWS) >> 1),
        ),
    )
    gid = (core * NB + blk) * 3 + region
    # sort by gather index within each (core, block, region) segment so each
    # gather's descriptor batch reads ascending HBM addresses
    order = np.lexsort((g_idx, gid))
    s_gid, s_idx = gid[order], g_idx[order]
    s_se, s_dl = se_all[order], dl[order]
    s_par = parity[order]
    # dedup: edges in the same segment with the same source share one slot
    # (the one-hot row then has several nonzero columns)
    new_slot = np.ones(E, bool)
    new_slot[1:] = (s_gid[1:] != s_gid[:-1]) | (s_idx[1:] != s_idx[:-1])
    slot_of_edge = np.cumsum(new_slot) - 1
    u_gid = s_gid[new_slot]
    u_idx = s_idx[new_slot]
    counts = np.bincount(u_gid, minlength=NCORES * NB * 3).reshape(NCORES, NB, 3)
    starts_f = np.concatenate([[0], np.cumsum(counts.reshape(-1))])[:-1]
    starts = starts_f.reshape(NCORES, NB, 3)
    pos_in_seg = slot_of_edge - starts_f[s_gid]
    T = np.ceil(counts.max(axis=0) / 128).astype(np.int64)  # [NB, 3]

    # step 4: tile layout. own region first; then per chunk: A tiles, B tiles.
    p1_tiles = [[] for _ in range(NB)]
    p2_tiles = [[] for _ in range(NB)]
    slot_start = np.zeros((NB, 3), np.int64)
    tbase = 0
    for b in range(NB):
        slot_start[b, 0] = tbase * 128
        p2_tiles[b] = list(range(tbase, tbase + T[b, 0]))
        tbase += T[b, 0]
    own_tiles = tbase
    chunks = []
    for k0 in range(0, NB, CHUNK_NB):
        blocks = list(range(k0, min(NB, k0 + CHUNK_NB)))
        TA = int(sum(T[b, 1] for b in blocks))
        TB = int(sum(T[b, 2] for b in blocks))
        off = tbase
        for b in blocks:
            slot_start[b, 1] = off * 128
            p1_tiles[b] = list(range(off, off + T[b, 1]))
            off += T[b, 1]
        for b in blocks:
            slot_start[b, 2] = off * 128
            p2_tiles[b] += list(range(off, off + T[b, 2]))
            off += T[b, 2]
        chunks.append((blocks, tbase, TA, TB))
        tbase = off
    ntiles = tbase
    nslot_e = ntiles * 128

    # slab layouts (consumption order): slab1 = A tiles, slab2 = own+B tiles
    pos1 = np.full(ntiles, -1, np.int64)
    pos2 = np.full(ntiles, -1, np.int64)
    chunk_tiles1, chunk_tiles2 = [], []
    n1 = n2 = 0
    for blocks, _, _, _ in chunks:
        c1 = c2 = 0
        for b in blocks:
            for t in p1_tiles[b]:
                pos1[t] = n1
                n1 += 1
                c1 += 1
            for t in p2_tiles[b]:
                pos2[t] = n2
                n2 += 1
                c2 += 1
        chunk_tiles1.append(c1)
        chunk_tiles2.append(c2)

    pos_idx = np.zeros(N, np.int64)
    ar = np.arange(NODES_PER)
    for g in range(N_GRAPHS):
        pos_idx[g * NODES_PER + perms[g]] = ar

    # per-edge global slot (in the tile layout) for slab building
    e_core = s_gid // (NB * 3)
    e_b = (s_gid // 3) % NB
    e_r = s_gid % 3
    e_slot = slot_start[e_b, e_r] + pos_in_seg
    u_core = u_gid // (NB * 3)
    u_b = (u_gid // 3) % NB
    u_r = u_gid % 3
    u_slot = slot_start[u_b, u_r] + (np.arange(len(u_gid)) - starts_f[u_gid])

    per_core = []
    for c in range(NCORES):
        idx_s = np.zeros(nslot_e, np.int64)
        m = u_core == c
        idx_s[u_slot[m]] = u_idx[m]
        me = e_core == c
        oldn = old_of[c * NPC : (c + 1) * NPC]
        lab_s = np.zeros(NSLOT, np.int64)
        lab_s[:NPC] = labels[oldn]
        pos_s = np.zeros(NSLOT, np.int64)
        pos_s[:NPC] = pos_idx[oldn]
        idxall = np.concatenate(
            [_wrap16(idx_s), _wrap16(lab_s), _wrap16(pos_s)], axis=1
        )
        per_core.append(
            dict(
                idxall=idxall,
                slot=e_slot[me].copy(),
                dl=s_dl[me].astype(np.int64),
                se=s_se[me].copy(),
                par=s_par[me].astype(np.int64),
            )
        )

    meta = dict(
        chunks=chunks, p1_tiles=p1_tiles, p2_tiles=p2_tiles, ntiles=ntiles,
        own_tiles=own_tiles, old_of=old_of, pos1=pos1, pos2=pos2,
        n1=n1, n2=n2, chunk_tiles1=chunk_tiles1, chunk_tiles2=chunk_tiles2,
    )
    return meta, per_core


def _build_cst(Ws, bs, w1, b1, w2, b2, w3, b3):
    """One [128, CSTW] f32 constant block -> single DMA, single dep."""
    cols = {}
    parts = []
    off = 0

    def add(name, arr):
        nonlocal off
        a = np.zeros((128, arr.shape[1]), np.float32)
        a[: arr.shape[0]] = arr
        cols[name] = off
        parts.append(a)
        off += arr.shape[1]

    add("ident", np.eye(128, dtype=np.float32))
    add("ident64", HSCALE * np.eye(128, dtype=np.float32))
    add("b4", np.stack([np.asarray(bs[l], np.float32) for l in range(NLAYERS)], 1))
    add("w1", np.asarray(w1, np.float32))
    add("b1", np.asarray(b1, np.float32)[:, None])
    add("w2", np.asarray(w2, np.float32))
    add("b2", np.asarray(b2, np.float32)[:, None])
    add("w3", np.asarray(w3, np.float32))
    add("b3", np.asarray(b3, np.float32)[:, None])
    return np.concatenate(parts, axis=1), cols


def _build_ohall(meta, slot, dl, se, par):
    """Precomputed one-hot (multi-hot after dedup) aggregation matrices.
    Each tile gets TWO [128,128] fp8 blocks (even|odd pair half), laid out
    as [128, ntiles*256] in per-phase consumption order."""
    import ml_dtypes

    tile = slot >> 7
    row = slot & 127
    p1 = meta["pos1"][tile]
    p2 = meta["pos2"][tile]
    slab1 = np.zeros((128, meta["n1"] * 256), np.float32)
    slab2 = np.zeros((128, meta["n2"] * 256), np.float32)
    m1 = p1 >= 0
    np.add.at(slab1, (row[m1], p1[m1] * 256 + par[m1] * 128 + dl[m1]), se[m1])
    m2 = p2 >= 0
    np.add.at(slab2, (row[m2], p2[m2] * 256 + par[m2] * 128 + dl[m2]), se[m2])
    return (
        slab1.astype(ml_dtypes.float8_e4m3),
        slab2.astype(ml_dtypes.float8_e4m3),
    )


def _build_nc(meta, cstw, ccols, idxw):
    chunks = meta["chunks"]
    p1_tiles = meta["p1_tiles"]
    p2_tiles = meta["p2_tiles"]
    ntiles = meta["ntiles"]
    own_tiles = meta["own_tiles"]
    tamax = max(TA for _, _, TA, _ in chunks)
    tbmax = max(TB for _, _, _, TB in chunks)
    LAG = int(os.environ.get("GCN_LAG", "6"))
    nchunks = len(chunks)
    LAG = min(LAG, nchunks - 1)
    SKIP_AG = os.environ.get("GCN_SKIP_AG", "") == "1"
    SKIP_GATHER = os.environ.get("GCN_SKIP_GATHER", "") == "1"
    SKIP_OH = os.environ.get("GCN_SKIP_OH", "") == "1"

    chunk_tiles1 = meta["chunk_tiles1"]
    chunk_tiles2 = meta["chunk_tiles2"]
    slab10 = np.concatenate([[0], np.cumsum(chunk_tiles1)])[:-1]
    slab20 = np.concatenate([[0], np.cumsum(chunk_tiles2)])[:-1]
    oh1max = max(chunk_tiles1)
    oh2max = max(chunk_tiles2)

    nc = bacc.Bacc(
        "TRN2", target_bir_lowering=False, debug=False, num_devices=NCORES,
        num_swdge_queues=4,
    )
    idxall = nc.dram_tensor("idxall", [128, idxw], I16, kind="ExternalInput").ap()
    cst_in = nc.dram_tensor("cst", [128, cstw], F32, kind="ExternalInput").ap()
    w4b_in = nc.dram_tensor(
        "w4b", [128, NLAYERS * D + 224], BF16, kind="ExternalInput"
    ).ap()
    idb_in = nc.dram_tensor("identb", [128, D], BF16, kind="ExternalInput").ap()
    idb64_in = nc.dram_tensor("identb64", [128, D], BF16, kind="ExternalInput").ap()
    emb_in = nc.dram_tensor("emb", [VOCAB, D], BF16, kind="ExternalInput").ap()
    pos_in = nc.dram_tensor("pos", [NODES_PER, D], BF16, kind="ExternalInput").ap()
    oh1_in = nc.dram_tensor(
        "ohall1", [128, meta["n1"] * 256], F8, kind="ExternalInput"
    ).ap()
    oh2_in = nc.dram_tensor(
        "ohall2", [128, meta["n2"] * 256], F8, kind="ExternalInput"
    ).ap()
    out_d = nc.dram_tensor("out", [NPC, D], F32, kind="ExternalOutput").ap()

    Relu = mybir.ActivationFunctionType.Relu
    Ident = mybir.ActivationFunctionType.Identity

    # round-robin SWDGE queue assignment: queue q runs on Q7 core pair
    # (2q, 2q+1), so consecutive gathers on different queues overlap their
    # descriptor generation.
    _gq = [0]

    def next_q():
        q = _gq[0]
        _gq[0] = (q + 1) % 4
        return q

    with tile.TileContext(nc) as tc:
        with (
            tc.tile_pool(name="persist", bufs=1) as pp,
            tc.tile_pool(name="gA", bufs=2) as gapool,
            tc.tile_pool(name="gB", bufs=2) as gbpool,
            tc.tile_pool(name="gown", bufs=1) as gopool,
            tc.tile_pool(name="oh1", bufs=2) as oh1pool,
            tc.tile_pool(name="oh2", bufs=2) as oh2pool,
            tc.tile_pool(name="mt", bufs=2) as mtpool,
            tc.tile_pool(name="c2", bufs=2) as c2pool,
            tc.tile_pool(name="zr", bufs=2) as zrpool,
            tc.tile_pool(name="hb", bufs=4) as hbpool,
            tc.tile_pool(name="ro", bufs=2) as ropool,
            tc.tile_pool(name="psm", bufs=3, space="PSUM") as psm,
            tc.tile_pool(name="psz", bufs=2, space="PSUM") as psz,
            tc.tile_pool(name="pst", bufs=2, space="PSUM") as pst,
            tc.tile_pool(name="dram", bufs=1, space="DRAM") as dram,
        ):
            idx_t = pp.tile([128, idxw], I16, tag="idx")
            nc.sync.dma_start(idx_t[:], idxall[:])
            cst = pp.tile([128, cstw], F32, tag="cst")
            nc.sync.dma_start(cst[:], cst_in[:])
            hT = pp.tile([128, NSLOT], BF16, tag="hT")

            def cc(name, j=0, rows=128, w=1):
                return cst[0:rows, ccols[name] + j : ccols[name] + j + w]

            w4b = pp.tile([128, NLAYERS * D + 224], BF16, tag="w4b")
            nc.sync.dma_start(w4b[:], w4b_in[:])
            identb = pp.tile([128, D], BF16, tag="identb")
            nc.sync.dma_start(identb[:], idb_in[:])
            identb64 = pp.tile([128, D], BF16, tag="identb64")
            nc.sync.dma_start(identb64[:], idb64_in[:])
            W4b_ap_all = w4b
            ident_ap = cc("ident", w=128)
            ident64_ap = cc("ident64", w=128)

            hgA = [
                dram.tile(
                    [NAP, 256], F8, tag=f"hgA{g}", name=f"hgA{g}",
                    addr_space="Shared",
                )
                for g in range(NLAYERS)
            ]
            hgB = [
                dram.tile(
                    [NBP, 256], F8, tag=f"hgB{g}", name=f"hgB{g}",
                    addr_space="Shared",
                )
                for g in range(NLAYERS)
            ]
            hgb = dram.tile([NPC // 2, 256], F8, tag="hgb")

            def ag1(gen):
                if SKIP_AG:
                    return
                nc.gpsimd.collective_compute(
                    "AllGather",
                    mybir.AluOpType.bypass,
                    replica_groups=[list(range(NCORES))],
                    ins=[hgb[0:APAIRS, :]],
                    outs=[hgA[gen].opt()],
                )

            def ag2(gen):
                if SKIP_AG:
                    return
                nc.gpsimd.collective_compute(
                    "AllGather",
                    mybir.AluOpType.bypass,
                    replica_groups=[list(range(NCORES))],
                    ins=[hgb[APAIRS : NPC // 2, :]],
                    outs=[hgB[gen].opt()],
                )

            def wb_rows(b):
                rows = LAST_ROWS if b == NB - 1 else 128
                return rows, hgb[b * 64 : b * 64 + rows // 2, :]

            def readout(off, cols):
                # MLP head 128->64->32->128 on hT[:, off:off+cols]
                W0 = NLAYERS * D
                p1 = psz.tile([64, 512], F32, tag="pz", name="ro_p1")
                nc.tensor.matmul(
                    p1[:, 0:cols], w4b[0:128, W0 : W0 + 64],
                    hT[:, off : off + cols],
                    start=True, stop=True,
                )
                x1 = ropool.tile([64, 512], BF16, tag="x1")
                nc.scalar.activation(
                    x1[:, 0:cols], p1[:, 0:cols], Relu, bias=cc("b1", rows=64)
                )
                p2 = psz.tile([32, 512], F32, tag="pz", name="ro_p2")
                nc.tensor.matmul(
                    p2[:, 0:cols], w4b[0:64, W0 + 64 : W0 + 96], x1[:, 0:cols],
                    start=True, stop=True,
                )
                x2 = ropool.tile([32, 512], BF16, tag="x2")
                nc.scalar.activation(
                    x2[:, 0:cols], p2[:, 0:cols], Relu, bias=cc("b2", rows=32)
                )
                p3 = psz.tile([128, 512], F32, tag="pz", name="ro_p3")
                nc.tensor.matmul(
                    p3[:, 0:cols], w4b[0:32, W0 + 96 : W0 + 224], x2[:, 0:cols],
                    start=True, stop=True,
                )
                x3 = ropool.tile([128, 512], F32, tag="x3")
                nc.scalar.activation(
                    x3[:, 0:cols], p3[:, 0:cols], Ident, bias=cc("b3")
                )
                for j in range(0, cols, 128):
                    b = (off + j) // 128
                    rows = LAST_ROWS if b == NB - 1 else 128
                    pt = pst.tile([128, 128], F32, tag="pt")
                    nc.tensor.transpose(pt[:], x3[:, j : j + 128], ident_ap)
                    ob = hbpool.tile([128, 128], F32, tag="hb")
                    nc.scalar.copy(ob[:], pt[:])
                    nc.sync.dma_start(
                        out_d[b * 128 : b * 128 + rows, :], ob[0:rows, :]
                    )

            # ---- setup: h0 = emb[labels] + pos_table[inv_perm] (bf16) ----
            setup_cm = tc.tile_pool(name="setup", bufs=1)
            sup = setup_cm.__enter__()
            ge = gopool.tile([128, NSLOT], BF16, tag="gown")
            hb0 = sup.tile([128, NSLOT], BF16, tag="hb0")
            e0 = ntiles * 8
            nc.gpsimd.dma_gather(
                ge[:, 0:NSLOT].rearrange("p (t e) -> p t e", e=D),
                emb_in[:, :],
                idx_t[:, e0 : e0 + NSLOT // 16],
                NSLOT, NSLOT, D, single_packet=False, queue_num=next_q(),
            )
            nc.gpsimd.dma_gather(
                hb0[:].rearrange("p (t e) -> p t e", e=D),
                pos_in[:, :],
                idx_t[:, e0 + NSLOT // 16 : e0 + 2 * (NSLOT // 16)],
                NSLOT, NSLOT, D, single_packet=False, queue_num=next_q(),
            )
            nc.vector.tensor_add(hb0[:], ge[:, 0:NSLOT], hb0[:])
            hb0s = sup.tile([128, NSLOT], F8, tag="hb0s")
            nc.scalar.activation(hb0s[:], hb0[:], Ident, scale=HSCALE)
            for b in range(NB):
                rows, dst = wb_rows(b)
                nc.sync.dma_start(dst, hb0s[0:rows, b * 128 : (b + 1) * 128])
            for b in range(NB):
                ptb = pst.tile([128, 128], BF16, tag="pt")
                nc.tensor.transpose(
                    ptb[:], hb0[:, b * 128 : (b + 1) * 128], identb[:]
                )
                nc.scalar.copy(hT[:, b * 128 : (b + 1) * 128], ptb[:])
            ag1(0)
            ag2(0)
            setup_cm.__exit__(None, None, None)
            mtp_cm = tc.tile_pool(name="mtp", bufs=LAG + 2)
            mtppool = mtp_cm.__enter__()

            # ---- GCN layers ----
            for l in range(int(os.environ.get("GCN_NLAYERS", NLAYERS))):
                par = l
                nlayers_run = int(os.environ.get("GCN_NLAYERS", NLAYERS))
                g_own = gopool.tile([128, own_tiles * 256], F8, tag="gown")
                for ot in range(0, own_tiles, 32):
                    if SKIP_GATHER:
                        break
                    on = min(32, own_tiles - ot)
                    nc.gpsimd.dma_gather(
                        g_own[:, ot * 256 : (ot + on) * 256].rearrange(
                            "p (t e) -> p t e", e=256
                        ),
                        hgb[:, :],
                        idx_t[:, ot * 8 : (ot + on) * 8],
                        on * 128, on * 128, 256, single_packet=False,
                        queue_num=next_q(),
                    )
                # phase 1: own+A aggregation per block (partials to SBUF);
                # phase 2 lags LAG chunks behind, adds the B contribution and
                # finishes the block (W matmul, residual, writeback). The lag
                # hides the previous layer's AG2 transfer.
                gA_t, gB_t = [None] * nchunks, [None] * nchunks

                def emit_A(k):
                    blocks, t0, TA, TB = chunks[k]
                    if TA == 0:
                        return
                    gA_t[k] = gapool.tile([128, tamax * 256], F8, tag="gA", name=f"gA_l{l}_{k}")
                    for h0, h1 in ((0, TA // 2), (TA // 2, TA)):
                        if h1 == h0 or SKIP_GATHER:
                            continue
                        nc.gpsimd.dma_gather(
                            gA_t[k][:, h0 * 256 : h1 * 256].rearrange(
                                "p (t e) -> p t e", e=256
                            ),
                            hgA[par][:, :],
                            idx_t[:, (t0 + h0) * 8 : (t0 + h1) * 8],
                            (h1 - h0) * 128, (h1 - h0) * 128, 256,
                            single_packet=False, queue_num=next_q(),
                        )

                def emit_B(k):
                    blocks, t0, TA, TB = chunks[k]
                    if TB == 0:
                        return
                    gB_t[k] = gbpool.tile([128, tbmax * 256], F8, tag="gB", name=f"gB_l{l}_{k}")
                    for h0, h1 in ((0, TB // 2), (TB // 2, TB)):
                        if h1 == h0 or SKIP_GATHER:
                            continue
                        nc.gpsimd.dma_gather(
                            gB_t[k][:, h0 * 256 : h1 * 256].rearrange(
                                "p (t e) -> p t e", e=256
                            ),
                            hgB[par][:, :],
                            idx_t[:, (t0 + TA + h0) * 8 : (t0 + TA + h1) * 8],
                            (h1 - h0) * 128, (h1 - h0) * 128, 256,
                            single_packet=False, queue_num=next_q(),
                        )

                oh1_t = [None] * nchunks
                oh2_t = [None] * nchunks

                def emit_oh1(k):
                    oh1_t[k] = oh1pool.tile(
                        [128, oh1max * 256], F8, tag="oh1", name=f"oh1_l{l}_{k}"
                    )
                    if SKIP_OH:
                        return
                    c0 = int(slab10[k]) * 256
                    cw = chunk_tiles1[k] * 256
                    nc.sync.dma_start(oh1_t[k][:, 0:cw], oh1_in[:, c0 : c0 + cw])

                def emit_oh2(k):
                    if chunk_tiles2[k] == 0:
                        return
                    oh2_t[k] = oh2pool.tile(
                        [128, oh2max * 256], F8, tag="oh2", name=f"oh2_l{l}_{k}"
                    )
                    if SKIP_OH:
                        return
                    c0 = int(slab20[k]) * 256
                    cw = chunk_tiles2[k] * 256
                    nc.sync.dma_start(oh2_t[k][:, 0:cw], oh2_in[:, c0 : c0 + cw])

                mTp_t = [None] * nchunks

                def phase1(k):
                    blocks, t0, TA, TB = chunks[k]
                    mTp_t[k] = mtppool.tile(
                        [128, 512], F32, tag="mTp", name=f"mTp_l{l}_{k}"
                    )
                    pos = 0
                    for j, b in enumerate(blocks):
                        tl = p1_tiles[b]
                        if not tl:
                            nc.vector.memset(
                                mTp_t[k][:, j * 128 : (j + 1) * 128], 0.0
                            )
                            continue
                        pm = psm.tile([128, 128], F32, tag="pm")
                        for i, t in enumerate(tl):
                            for h in (0, 1):
                                oh = oh1_t[k][
                                    :, pos * 256 + h * 128 : pos * 256 + (h + 1) * 128
                                ]
                                lhs = gA_t[k][
                                    :,
                                    (t - t0) * 256 + h * 128
                                    : (t - t0) * 256 + (h + 1) * 128,
                                ]
                                nc.tensor.matmul(
                                    pm[:], lhs, oh,
                                    start=(i == 0 and h == 0),
                                    stop=(i == len(tl) - 1 and h == 1),
                                )
                            pos += 1
                        nc.scalar.copy(mTp_t[k][:, j * 128 : (j + 1) * 128], pm[:])

                def phase2(k):
                    blocks, t0, TA, TB = chunks[k]
                    mT = mtpool.tile([128, 512], BF16, tag="mT")
                    pos = 0
                    for j, b in enumerate(blocks):
                        tl = p2_tiles[b]
                        jc = slice(j * 128, (j + 1) * 128)
                        if tl:
                            pm = psm.tile([128, 128], F32, tag="pm")
                            for i, t in enumerate(tl):
                                for h in (0, 1):
                                    oh = oh2_t[k][
                                        :,
                                        pos * 256 + h * 128
                                        : pos * 256 + (h + 1) * 128,
                                    ]
                                    if t < own_tiles:
                                        lhs = g_own[
                                            :, t * 256 + h * 128 : t * 256 + (h + 1) * 128
                                        ]
                                    else:
                                        lhs = gB_t[k][
                                            :,
                                            (t - t0 - TA) * 256 + h * 128
                                            : (t - t0 - TA) * 256 + (h + 1) * 128,
                                        ]
                                    nc.tensor.matmul(
                                        pm[:], lhs, oh,
                                        start=(i == 0 and h == 0),
                                        stop=(i == len(tl) - 1 and h == 1),
                                    )
                                pos += 1
                            c2 = c2pool.tile([128, 128], F32, tag="c2")
                            nc.scalar.copy(c2[:], pm[:])
                            nc.vector.tensor_add(
                                mT[:, jc], mTp_t[k][:, jc], c2[:]
                            )
                        else:
                            nc.scalar.copy(mT[:, jc], mTp_t[k][:, jc])
                    cols = len(blocks) * 128
                    pz = psz.tile([128, 512], F32, tag="pz")
                    nc.tensor.matmul(
                        pz[:, 0:cols],
                        w4b[:, l * 128 : (l + 1) * 128],
                        mT[:, 0:cols],
                        start=True, stop=True,
                    )
                    zr = zrpool.tile([128, 512], BF16, tag="zr")
                    nc.scalar.activation(
                        zr[:, 0:cols], pz[:, 0:cols], Relu, bias=cc("b4", l)
                    )
                    c0 = blocks[0] * 128
                    nc.vector.tensor_add(
                        hT[:, c0 : c0 + cols], hT[:, c0 : c0 + cols], zr[:, 0:cols]
                    )
                    if l < nlayers_run - 1:
                        for b in blocks:
                            rows, dst = wb_rows(b)
                            pt = pst.tile([128, 128], BF16, tag="pt")
                            nc.tensor.transpose(
                                pt[:], hT[:, b * 128 : (b + 1) * 128], identb[:]
                            )
                            hb = hbpool.tile([128, 128], F8, tag="hbw")
                            nc.scalar.activation(
                                hb[:], pt[:], Ident, scale=HSCALE
                            )
                            nc.sync.dma_start(dst, hb[0:rows, :])
                        if blocks[-1] == HBLK - 1:
                            ag1(l + 1)
                    else:
                        readout(c0, cols)

                for k in range(nchunks):
                    emit_A(k)
                emit_oh1(0)
                emit_oh1(1)
                oh2_next = [0]

                def ensure_oh2(upto):
                    while oh2_next[0] <= min(upto, nchunks - 1):
                        emit_oh2(oh2_next[0])
                        oh2_next[0] += 1

                for k in range(nchunks):
                    emit_B(k)
                    if k + 2 < nchunks:
                        emit_oh1(k + 2)
                    phase1(k)
                    k2 = k - LAG
                    if k2 >= 0:
                        ensure_oh2(k2 + 1)
                        phase2(k2)
                for k2 in range(max(0, nchunks - LAG), nchunks):
                    ensure_oh2(k2 + 1)
                    phase2(k2)
                if l < nlayers_run - 1:
                    ag2(l + 1)
            mtp_cm.__exit__(None, None, None)

    nc.compile()
    return nc


last_results = None


def kernel(labels, src, dst, perms, emb, Ws, bs, w1, b1, w2, b2, w3, b3):
    global last_results
    meta, per_core = _preprocess(labels, src, dst, perms)
    cst0, ccols = _build_cst(Ws, bs, w1, b1, w2, b2, w3, b3)
    key = (meta["ntiles"], meta["own_tiles"], meta["n1"], meta["n2"],
           os.environ.get("GCN_NLAYERS", ""), os.environ.get("GCN_LAG", ""),
           os.environ.get("GCN_SKIP_AG", ""),
           os.environ.get("GCN_SKIP_GATHER", ""),
           os.environ.get("GCN_SKIP_OH", ""),
           tuple(t for _, t, _, _ in meta["chunks"]))
    if key not in _cache:
        _cache[key] = _build_nc(
            meta, cst0.shape[1], ccols, per_core[0]["idxall"].shape[1]
        )
    nc = _cache[key]

    import ml_dtypes
    emb_np = np.asarray(emb, np.float32).astype(ml_dtypes.bfloat16)
    pos_np = _pos_table().astype(ml_dtypes.bfloat16)
    idb_np = np.eye(128, dtype=ml_dtypes.bfloat16)
    idb64_np = (HSCALE * np.eye(128, dtype=np.float32)).astype(ml_dtypes.bfloat16)
    w4b_parts = [
        np.concatenate(
            [np.asarray(Ws[l], np.float32) for l in range(NLAYERS)], 1
        )
        / HSCALE
    ]
    for wmat, rows in ((w1, 128), (w2, 64), (w3, 32)):
        wp = np.zeros((128, np.asarray(wmat).shape[1]), np.float32)
        wp[:rows] = np.asarray(wmat, np.float32)
        w4b_parts.append(wp)
    w4b_np = np.concatenate(w4b_parts, 1).astype(ml_dtypes.bfloat16)
    in_maps = []
    for c in range(NCORES):
        oh1_c, oh2_c = _build_ohall(
            meta, per_core[c]["slot"], per_core[c]["dl"], per_core[c]["se"],
            per_core[c]["par"],
        )
        in_maps.append(
            dict(idxall=per_core[c]["idxall"], cst=cst0, emb=emb_np, pos=pos_np,
                 w4b=w4b_np, identb=idb_np, identb64=idb64_np,
                 ohall1=oh1_c, ohall2=oh2_c)
        )
    res = run_bass_kernel_spmd(nc, in_maps, core_ids=list(range(NCORES)))
    last_results = res
    cat = np.concatenate([res.results[c]["out"] for c in range(NCORES)], axis=0)
    out = np.empty_like(cat)
    out[meta["old_of"]] = cat
    return out



# revision 40
# speedup vs baseline: 1.0980x; 1.0980x over previous
"""Trainium2 Bass kernel for a 4-layer GCN (nn_GCNNet).

Strategy (8 NeuronCores, SPMD single NEFF):
  - Core c owns the contiguous node range [c*6250, (c+1)*6250) and all edges
    whose dst falls in that range (edge sharding by destination).
  - Node features h live transposed in SBUF as hT [128 d, 6250 nodes] f32.
  - Per GCN layer: every core gathers h[src] rows for its edges from a
    replicated DRAM copy of h (dma_gather, 256B bf16 rows), aggregates them
    into m^T per 128-dst-node block with one-hot matmuls accumulating in
    PSUM (the fp8 one-hot carries the symmetric-norm coefficient per edge
    and is precomputed host-side, streamed from DRAM), applies the layer
    weight as a [128x128] @ [128x512] matmul, relu+bias on the scalar
    engine, residual-adds into hT, and publishes its updated node shard via
    Shared-output AllGather so every core has the full h for the next layer.
  - dma_gather indices are int16, so the gather source is addressed as two
    ~25k-row halves (A = first 24 blocks per core, B = rest), published by
    two separate AllGathers. Gather descriptor generation runs on Q7 core
    pairs selected by the SWDGE queue number, so gathers are striped
    round-robin over queues 0-3 (and split in half) to overlap the
    per-index descriptor-generation cost 4 ways.
  - Each layer runs in two phases: phase 1 aggregates own+A-sourced edge
    tiles for every block (partials parked in SBUF); phase 2 lags LAG
    chunks behind, adds the B-sourced tiles, applies W, the residual, and
    the writeback. The lag keeps compute flowing while the previous
    layer's AllGather of the B half is still in flight.
  - Edges within each (block, region) segment are sorted by gather index
    (ascending HBM addresses per descriptor batch) and edges sharing a
    source collapse into one gathered slot with a multi-hot column.
  - MLP readout (128->64->32->128) runs on the transposed features, then
    tiles are transposed back via the PE and DMA'd out.

Host-side work is limited to graph preprocessing: sharding/sorting edges,
padding, building index streams and fp8 one-hot slabs, degree counts and
the norm coefficients isq_src[src]*isq_dst[dst] (pure functions of the
integer edge lists), plus the constant sinusoidal position table. All
tensor math (embedding lookup, aggregation, matmuls, activations,
residuals, readout) runs on device.
"""

import os
import sys

sys.path.insert(0, "/opt/trn_rl_repo")

import math

import numpy as np

import concourse.bacc as bacc
import concourse.bass as bass
import concourse.mybir as mybir
import concourse.tile as tile
from concourse.bass_utils import run_bass_kernel_spmd

# Problem constants (hardcoded per contest rules).
N_GRAPHS = 25
NODES_PER = 2000
N = N_GRAPHS * NODES_PER          # 50000
E = 800000
D = 128
VOCAB = 30
NLAYERS = 4
NCORES = 8
NPC = N // NCORES                 # 6250 nodes per core
HBLK = 24                         # blocks per AG1 prefix ("A" half)
AROWS = HBLK * 128                # 3072 rows per core in the A half
BROWS = NPC - AROWS               # 3178 rows per core in the B half
NA = NCORES * AROWS               # 24576 rows in hgA
NB_ROWS = NCORES * BROWS          # 25424 rows in hgB
NB = (NPC + 127) // 128           # 49 dst blocks / node tiles per core
LAST_ROWS = NPC - 128 * (NB - 1)  # 106 valid rows in the last tile
NSLOT = NB * 128                  # 6272 padded node slots
CHUNK_NB = 4                      # dst blocks per gather chunk (= W-matmul group)

F32 = mybir.dt.float32
BF16 = mybir.dt.bfloat16
F8 = mybir.dt.float8e4
I16 = mybir.dt.int16

_cache = {}


def _pos_table():
    pos = (np.arange(NODES_PER, dtype=np.float64) + 1.0)[:, None]
    div = np.exp(np.arange(0, D, 2, dtype=np.float64) * (-math.log(10000.0) / D))
    ang = pos * div
    tab = np.stack([np.sin(ang), np.cos(ang)], axis=-1).reshape(NODES_PER, D)
    return tab.astype(np.float32)


def _wrap16(stream):
    """int16 index stream -> [128, len/16] SBUF layout (16-partition wrap,
    replicated to all 8 gpsimd cores)."""
    v = stream.reshape(-1, 16).T  # [16, cols]
    return np.tile(v, (8, 1)).astype(np.int16)


def _balance_partition(deg_vec):
    """Assign nodes to 8 cores (6250 each), balancing total in-degree.
    Returns old_of_new: new label -> old node id."""
    order = np.argsort(-deg_vec, kind="stable")
    loads = np.zeros(NCORES)
    counts = np.zeros(NCORES, np.int64)
    assign = np.empty(N, np.int64)
    for v in order:
        c = int(np.argmin(np.where(counts < NPC, loads, np.inf)))
        assign[v] = c
        loads[c] += deg_vec[v]
        counts[c] += 1
    old_of = np.empty(N, np.int64)
    pos = np.zeros(NCORES, np.int64)
    # blocks are packed later; here order within a core is provisional
    for v in np.arange(N):
        c = assign[v]
        old_of[c * NPC + pos[c]] = v
        pos[c] += 1
    return assign, old_of


def _caps2(nfat=16):
    c = np.tile(np.array([256, 1792], np.int64), (NB, 1))
    c[:nfat] = (384, 2048)
    return c


def _caps3(nfat):
    c = np.tile(np.array([256, 896, 896], np.int64), (NB, 1))
    c[:nfat] = (384, 1024, 1024)
    return c


def _pack_blocks(nodes_old, wmat, caps, init_members=None):
    if caps.ndim == 1:
        caps = np.tile(caps, (NB, 1))
    """Pack one core's 6250 nodes into 49 blocks (last=106 nodes) under
    per-block edge quotas; lowest-index-first so fill patterns align across
    cores (tile counts are cross-core maxes)."""
    order = np.argsort(-wmat.sum(1), kind="stable")
    ncaps = caps.shape[-1]
    if init_members is not None:
        members = [list(m) for m in init_members]
        node_w = {int(nodes_old[i]): wmat[i] for i in range(len(nodes_old))}
        loads = np.zeros((NB, ncaps), np.int64)
        for b in range(NB):
            for v in members[b]:
                loads[b] += node_w[v]
        return _refine(members, node_w, loads, caps)
    loads = np.zeros((NB, ncaps), np.int64)
    counts = np.zeros(NB, np.int64)
    block_cap = np.full(NB, 128, np.int64)
    block_cap[NB - 1] = LAST_ROWS
    members = [[] for _ in range(NB)]
    for i in order:
        v = nodes_old[i]
        wv = wmat[i]
        fits = (counts[:-1] < block_cap[:-1]) & np.all(
            loads[:-1] + wv[None, :] <= caps[:-1], axis=1
        )
        if fits.any():
            b = int(np.argmax(fits))
        elif counts[NB - 1] < block_cap[NB - 1]:
            b = NB - 1
        else:
            over = ((loads[:-1] + wv[None, :]) / caps[:-1]).max(1)
            over[counts[:-1] >= block_cap[:-1]] = np.inf
            b = NB - 2 - int(np.argmin(over[::-1]))
        members[b].append(v)
        loads[b] += wv
        counts[b] += 1
    assert all(len(members[b]) == block_cap[b] for b in range(NB))
    node_w = {int(nodes_old[i]): wmat[i] for i in range(len(nodes_old))}
    loads = np.zeros((NB, wmat.shape[1]), np.int64)
    for b in range(NB):
        for v in members[b]:
            loads[b] += node_w[v]
    return _refine(members, node_w, loads, caps)


def _refine(members, node_w, loads, caps):
    for _ in range(4000):
        over = (loads[:-1] - caps[:-1]).max(1)
        b = int(np.argmax(over))
        if over[b] <= 0:
            break
        d = int(np.argmax(loads[b] - caps[b]))
        # candidate donors: big-w[d] nodes of b; receivers: slackiest block
        done = False
        for b2 in np.argsort(-(caps[:-1, d] - loads[:-1, d]))[:6]:
            if b2 == b:
                continue
            mw = [node_w[v][d] for v in members[b]]
            for ui in np.argsort(mw)[::-1][:8]:
                u = members[b][int(ui)]
                wu = node_w[u]
                for vi, v in enumerate(members[b2][:64]):
                    wv = node_w[v]
                    delta = wu - wv
                    if delta[d] <= 0:
                        continue
                    nb = loads[b] - delta
                    nb2 = loads[b2] + delta
                    if (nb2 <= caps[b2]).all() and (nb - caps[b]).max() < over[b]:
                        members[b][int(ui)] = v
                        members[b2][vi] = u
                        loads[b] = nb
                        loads[b2] = nb2
                        done = True
                        break
                if done:
                    break
            if done:
                break
        if not done:
            break
    return members


def _label_from_blocks(assign, blocks_per_core):
    old_of = np.empty(N, np.int64)
    p = 0
    for c in range(NCORES):
        for b in range(NB):
            for v in blocks_per_core[c][b]:
                old_of[p] = v
                p += 1
    newid = np.empty(N, np.int64)
    newid[old_of] = np.arange(N)
    return old_of, newid


def _preprocess(labels, src, dst, perms):
    """Relabel/shard/sort/pad edges; build per-core device input arrays."""
    src = np.asarray(src).astype(np.int64)
    dst = np.asarray(dst).astype(np.int64)
    labels = np.asarray(labels).astype(np.int64)
    perms = np.asarray(perms).astype(np.int64)

    deg_out = np.bincount(src, minlength=N)
    deg_in = np.bincount(dst, minlength=N)
    isq_src = (np.maximum(deg_out, 1) ** -0.5).astype(np.float32)
    isq_dst = (np.maximum(deg_in, 1) ** -0.5).astype(np.float32)
    se_all = (isq_src[src] * isq_dst[dst]).astype(np.float32)

    # step 1: balanced core assignment (by in-degree)
    assign, _ = _balance_partition(deg_in.astype(np.float64))
    src_core = assign[src]
    own_edge = src_core == assign[dst]
    d_own = np.bincount(dst[own_edge], minlength=N)
    d_no = np.bincount(dst[~own_edge], minlength=N)

    # step 2 round 1: pack by (own, nonown) to get provisional labels
    blocks1 = []
    for c in range(NCORES):
        nodes_c = np.where(assign == c)[0]
        w = np.stack([d_own[nodes_c], d_no[nodes_c]], 1)
        blocks1.append(_pack_blocks(nodes_c, w, _caps2()))
    old_of, newid = _label_from_blocks(assign, blocks1)

    # step 2 round 2: A = src in first HBLK blocks of its core; repack with
    # (own, A, B) quotas using round-1 membership as the estimate
    in_a = (newid[src] % NPC) < AROWS
    d_a = np.bincount(dst[(~own_edge) & in_a], minlength=N)
    d_b = np.bincount(dst[(~own_edge) & ~in_a], minlength=N)
    loads_ab = np.zeros((NCORES, 2), np.int64)
    for c in range(NCORES):
        m = assign == c
        loads_ab[c] = (d_a[m].sum(), d_b[m].sum())
    nfat = int(min(48, np.ceil((loads_ab.max() - 48 * 896) / 128) + 6))
    nfat = max(nfat, 0)
    blocks2 = []
    for c in range(NCORES):
        nodes_c = np.where(assign == c)[0]
        w = np.stack([d_own[nodes_c], d_a[nodes_c], d_b[nodes_c]], 1)
        blocks2.append(_pack_blocks(nodes_c, w, _caps3(nfat)))
    old_of, newid = _label_from_blocks(assign, blocks2)

    # round 3: one more iteration with refreshed A/B membership
    in_a = (newid[src] % NPC) < AROWS
    d_a = np.bincount(dst[(~own_edge) & in_a], minlength=N)
    d_b = np.bincount(dst[(~own_edge) & ~in_a], minlength=N)
    blocks3 = []
    for c in range(NCORES):
        nodes_c = np.where(assign == c)[0]
        w = np.stack([d_own[nodes_c], d_a[nodes_c], d_b[nodes_c]], 1)
        blocks3.append(
            _pack_blocks(nodes_c, w, _caps3(nfat), init_members=blocks2[c])
        )
    old_of, newid = _label_from_blocks(assign, blocks3)

    # round 4: refine once more against refreshed membership
    in_a = (newid[src] % NPC) < AROWS
    d_a = np.bincount(dst[(~own_edge) & in_a], minlength=N)
    d_b = np.bincount(dst[(~own_edge) & ~in_a], minlength=N)
    blocks4 = []
    for c in range(NCORES):
        nodes_c = np.where(assign == c)[0]
        w = np.stack([d_own[nodes_c], d_a[nodes_c], d_b[nodes_c]], 1)
        blocks4.append(
            _pack_blocks(nodes_c, w, _caps3(nfat), init_members=blocks3[c])
        )
    old_of, newid = _label_from_blocks(assign, blocks4)

    src_n = newid[src]
    dst_n = newid[dst]

    # step 3: edge grouping on FINAL labels
    core = dst_n // NPC
    dstloc = dst_n % NPC
    blk = dstloc >> 7
    dl = (dstloc & 127).astype(np.float32)
    src_core_n = src_n // NPC
    src_loc = src_n % NPC
    own = src_core_n == core
    in_a = src_loc < AROWS
    region = np.where(own, 0, np.where(in_a, 1, 2))
    # gather index per edge by region
    g_idx = np.where(
        own,
        src_loc,
        np.where(
            in_a,
            src_core_n * AROWS + src_loc,
            src_core_n * BROWS + (src_loc - AROWS),
        ),
    )
    gid = (core * NB + blk) * 3 + region
    # sort by gather index within each (core, block, region) segment so each
    # gather's descriptor batch reads ascending HBM addresses
    order = np.lexsort((g_idx, gid))
    s_gid, s_idx = gid[order], g_idx[order]
    s_se, s_dl = se_all[order], dl[order]
    # dedup: edges in the same segment with the same source share one slot
    # (the one-hot row then has several nonzero columns)
    new_slot = np.ones(E, bool)
    new_slot[1:] = (s_gid[1:] != s_gid[:-1]) | (s_idx[1:] != s_idx[:-1])
    slot_of_edge = np.cumsum(new_slot) - 1
    u_gid = s_gid[new_slot]
    u_idx = s_idx[new_slot]
    counts = np.bincount(u_gid, minlength=NCORES * NB * 3).reshape(NCORES, NB, 3)
    starts_f = np.concatenate([[0], np.cumsum(counts.reshape(-1))])[:-1]
    starts = starts_f.reshape(NCORES, NB, 3)
    pos_in_seg = slot_of_edge - starts_f[s_gid]
    T = np.ceil(counts.max(axis=0) / 128).astype(np.int64)  # [NB, 3]

    # step 4: tile layout. own region first; then per chunk: A tiles, B tiles.
    p1_tiles = [[] for _ in range(NB)]
    p2_tiles = [[] for _ in range(NB)]
    slot_start = np.zeros((NB, 3), np.int64)
    tbase = 0
    for b in range(NB):
        slot_start[b, 0] = tbase * 128
        p2_tiles[b] = list(range(tbase, tbase + T[b, 0]))
        tbase += T[b, 0]
    own_tiles = tbase
    chunks = []
    for k0 in range(0, NB, CHUNK_NB):
        blocks = list(range(k0, min(NB, k0 + CHUNK_NB)))
        TA = int(sum(T[b, 1] for b in blocks))
        TB = int(sum(T[b, 2] for b in blocks))
        off = tbase
        for b in blocks:
            slot_start[b, 1] = off * 128
            p1_tiles[b] = list(range(off, off + T[b, 1]))
            off += T[b, 1]
        for b in blocks:
            slot_start[b, 2] = off * 128
            p2_tiles[b] += list(range(off, off + T[b, 2]))
            off += T[b, 2]
        chunks.append((blocks, tbase, TA, TB))
        tbase = off
    ntiles = tbase
    nslot_e = ntiles * 128

    # slab layouts (consumption order): slab1 = A tiles, slab2 = own+B tiles
    pos1 = np.full(ntiles, -1, np.int64)
    pos2 = np.full(ntiles, -1, np.int64)
    chunk_tiles1, chunk_tiles2 = [], []
    n1 = n2 = 0
    for blocks, _, _, _ in chunks:
        c1 = c2 = 0
        for b in blocks:
            for t in p1_tiles[b]:
                pos1[t] = n1
                n1 += 1
                c1 += 1
            for t in p2_tiles[b]:
                pos2[t] = n2
                n2 += 1
                c2 += 1
        chunk_tiles1.append(c1)
        chunk_tiles2.append(c2)

    pos_idx = np.zeros(N, np.int64)
    ar = np.arange(NODES_PER)
    for g in range(N_GRAPHS):
        pos_idx[g * NODES_PER + perms[g]] = ar

    # per-edge global slot (in the tile layout) for slab building
    e_core = s_gid // (NB * 3)
    e_b = (s_gid // 3) % NB
    e_r = s_gid % 3
    e_slot = slot_start[e_b, e_r] + pos_in_seg
    u_core = u_gid // (NB * 3)
    u_b = (u_gid // 3) % NB
    u_r = u_gid % 3
    u_slot = slot_start[u_b, u_r] + (np.arange(len(u_gid)) - starts_f[u_gid])

    per_core = []
    for c in range(NCORES):
        idx_s = np.zeros(nslot_e, np.int64)
        m = u_core == c
        idx_s[u_slot[m]] = u_idx[m]
        me = e_core == c
        oldn = old_of[c * NPC : (c + 1) * NPC]
        lab_s = np.zeros(NSLOT, np.int64)
        lab_s[:NPC] = labels[oldn]
        pos_s = np.zeros(NSLOT, np.int64)
        pos_s[:NPC] = pos_idx[oldn]
        idxall = np.concatenate(
            [_wrap16(idx_s), _wrap16(lab_s), _wrap16(pos_s)], axis=1
        )
        per_core.append(
            dict(
                idxall=idxall,
                slot=e_slot[me].copy(),
                dl=s_dl[me].astype(np.int64),
                se=s_se[me].copy(),
            )
        )

    meta = dict(
        chunks=chunks, p1_tiles=p1_tiles, p2_tiles=p2_tiles, ntiles=ntiles,
        own_tiles=own_tiles, old_of=old_of, pos1=pos1, pos2=pos2,
        n1=n1, n2=n2, chunk_tiles1=chunk_tiles1, chunk_tiles2=chunk_tiles2,
    )
    return meta, per_core


def _build_cst(Ws, bs, w1, b1, w2, b2, w3, b3):
    """One [128, CSTW] f32 constant block -> single DMA, single dep."""
    cols = {}
    parts = []
    off = 0

    def add(name, arr):
        nonlocal off
        a = np.zeros((128, arr.shape[1]), np.float32)
        a[: arr.shape[0]] = arr
        cols[name] = off
        parts.append(a)
        off += arr.shape[1]

    add("ident", np.eye(128, dtype=np.float32))
    add("b4", np.stack([np.asarray(bs[l], np.float32) for l in range(NLAYERS)], 1))
    add("w1", np.asarray(w1, np.float32))
    add("b1", np.asarray(b1, np.float32)[:, None])
    add("w2", np.asarray(w2, np.float32))
    add("b2", np.asarray(b2, np.float32)[:, None])
    add("w3", np.asarray(w3, np.float32))
    add("b3", np.asarray(b3, np.float32)[:, None])
    return np.concatenate(parts, axis=1), cols


def _build_ohall(meta, slot, dl, se):
    """Precomputed one-hot (multi-hot after dedup) aggregation matrices,
    one [128,128] fp8 block per tile, in per-phase consumption order."""
    import ml_dtypes

    tile = slot >> 7
    row = slot & 127
    p1 = meta["pos1"][tile]
    p2 = meta["pos2"][tile]
    slab1 = np.zeros((128, meta["n1"] * 128), np.float32)
    slab2 = np.zeros((128, meta["n2"] * 128), np.float32)
    m1 = p1 >= 0
    np.add.at(slab1, (row[m1], p1[m1] * 128 + dl[m1]), se[m1])
    m2 = p2 >= 0
    np.add.at(slab2, (row[m2], p2[m2] * 128 + dl[m2]), se[m2])
    return (
        slab1.astype(ml_dtypes.float8_e4m3),
        slab2.astype(ml_dtypes.float8_e4m3),
    )


def _build_nc(meta, cstw, ccols, idxw):
    chunks = meta["chunks"]
    p1_tiles = meta["p1_tiles"]
    p2_tiles = meta["p2_tiles"]
    ntiles = meta["ntiles"]
    own_tiles = meta["own_tiles"]
    tamax = max(TA for _, _, TA, _ in chunks)
    tbmax = max(TB for _, _, _, TB in chunks)
    LAG = int(os.environ.get("GCN_LAG", "6"))
    nchunks = len(chunks)
    LAG = min(LAG, nchunks - 1)
    SKIP_AG = os.environ.get("GCN_SKIP_AG", "") == "1"
    SKIP_GATHER = os.environ.get("GCN_SKIP_GATHER", "") == "1"
    SKIP_OH = os.environ.get("GCN_SKIP_OH", "") == "1"

    chunk_tiles1 = meta["chunk_tiles1"]
    chunk_tiles2 = meta["chunk_tiles2"]
    slab10 = np.concatenate([[0], np.cumsum(chunk_tiles1)])[:-1]
    slab20 = np.concatenate([[0], np.cumsum(chunk_tiles2)])[:-1]
    oh1max = max(chunk_tiles1)
    oh2max = max(chunk_tiles2)

    nc = bacc.Bacc(
        "TRN2", target_bir_lowering=False, debug=False, num_devices=NCORES,
        num_swdge_queues=4,
    )
    idxall = nc.dram_tensor("idxall", [128, idxw], I16, kind="ExternalInput").ap()
    cst_in = nc.dram_tensor("cst", [128, cstw], F32, kind="ExternalInput").ap()
    w4b_in = nc.dram_tensor("w4b", [128, NLAYERS * D], BF16, kind="ExternalInput").ap()
    idb_in = nc.dram_tensor("identb", [128, D], BF16, kind="ExternalInput").ap()
    emb_in = nc.dram_tensor("emb", [VOCAB, D], BF16, kind="ExternalInput").ap()
    pos_in = nc.dram_tensor("pos", [NODES_PER, D], BF16, kind="ExternalInput").ap()
    oh1_in = nc.dram_tensor(
        "ohall1", [128, meta["n1"] * 128], F8, kind="ExternalInput"
    ).ap()
    oh2_in = nc.dram_tensor(
        "ohall2", [128, meta["n2"] * 128], F8, kind="ExternalInput"
    ).ap()
    out_d = nc.dram_tensor("out", [NPC, D], F32, kind="ExternalOutput").ap()

    Relu = mybir.ActivationFunctionType.Relu
    Ident = mybir.ActivationFunctionType.Identity

    # round-robin SWDGE queue assignment: queue q runs on Q7 core pair
    # (2q, 2q+1), so consecutive gathers on different queues overlap their
    # descriptor generation.
    _gq = [0]

    def next_q():
        q = _gq[0]
        _gq[0] = (q + 1) % 4
        return q

    with tile.TileContext(nc) as tc:
        with (
            tc.tile_pool(name="persist", bufs=1) as pp,
            tc.tile_pool(name="gA", bufs=3) as gapool,
            tc.tile_pool(name="gB", bufs=3) as gbpool,
            tc.tile_pool(name="gown", bufs=1) as gopool,
            tc.tile_pool(name="oh1", bufs=3) as oh1pool,
            tc.tile_pool(name="oh2", bufs=3) as oh2pool,
            tc.tile_pool(name="mtp", bufs=LAG + 2) as mtppool,
            tc.tile_pool(name="mt", bufs=2) as mtpool,
            tc.tile_pool(name="c2", bufs=2) as c2pool,
            tc.tile_pool(name="zr", bufs=2) as zrpool,
            tc.tile_pool(name="hb", bufs=4) as hbpool,
            tc.tile_pool(name="ro", bufs=2) as ropool,
            tc.tile_pool(name="psm", bufs=3, space="PSUM") as psm,
            tc.tile_pool(name="psz", bufs=2, space="PSUM") as psz,
            tc.tile_pool(name="pst", bufs=2, space="PSUM") as pst,
            tc.tile_pool(name="dram", bufs=1, space="DRAM") as dram,
        ):
            idx_t = pp.tile([128, idxw], I16, tag="idx")
            nc.sync.dma_start(idx_t[:], idxall[:])
            cst = pp.tile([128, cstw], F32, tag="cst")
            nc.sync.dma_start(cst[:], cst_in[:])
            hT = pp.tile([128, NSLOT], F32, tag="hT")

            def cc(name, j=0, rows=128, w=1):
                return cst[0:rows, ccols[name] + j : ccols[name] + j + w]

            w4b = pp.tile([128, NLAYERS * D], BF16, tag="w4b")
            nc.sync.dma_start(w4b[:], w4b_in[:])
            identb = pp.tile([128, D], BF16, tag="identb")
            nc.sync.dma_start(identb[:], idb_in[:])
            W4b_ap_all = w4b
            ident_ap = cc("ident", w=128)

            hgA = [
                dram.tile(
                    [NA, D], BF16, tag=f"hgA{g}", name=f"hgA{g}",
                    addr_space="Shared",
                )
                for g in range(NLAYERS)
            ]
            hgB = [
                dram.tile(
                    [NB_ROWS, D], BF16, tag=f"hgB{g}", name=f"hgB{g}",
                    addr_space="Shared",
                )
                for g in range(NLAYERS)
            ]
            hgb = dram.tile([NPC, D], BF16, tag="hgb")

            def ag1(gen):
                if SKIP_AG:
                    return
                nc.gpsimd.collective_compute(
                    "AllGather",
                    mybir.AluOpType.bypass,
                    replica_groups=[list(range(NCORES))],
                    ins=[hgb[0:AROWS, :]],
                    outs=[hgA[gen].opt()],
                )

            def ag2(gen):
                if SKIP_AG:
                    return
                nc.gpsimd.collective_compute(
                    "AllGather",
                    mybir.AluOpType.bypass,
                    replica_groups=[list(range(NCORES))],
                    ins=[hgb[AROWS:, :]],
                    outs=[hgB[gen].opt()],
                )

            def writeback(src_ap_of_tile):
                for b in range(NB):
                    rows = LAST_ROWS if b == NB - 1 else 128
                    hb = hbpool.tile([128, 128], F32, tag="hb")
                    nc.scalar.copy(hb[:], src_ap_of_tile(b))
                    nc.sync.dma_start(hgb[b * 128 : b * 128 + rows, :], hb[0:rows, :])

            def readout(off, cols):
                # MLP head 128->64->32->128 on hT[:, off:off+cols]
                p1 = psz.tile([64, 512], F32, tag="pz", name="ro_p1")
                nc.tensor.matmul(
                    p1[:, 0:cols], cc("w1", rows=128, w=64),
                    hT[:, off : off + cols],
                    start=True, stop=True,
                )
                x1 = ropool.tile([64, 512], F32, tag="x1")
                nc.scalar.activation(
                    x1[:, 0:cols], p1[:, 0:cols], Relu, bias=cc("b1", rows=64)
                )
                p2 = psz.tile([32, 512], F32, tag="pz", name="ro_p2")
                nc.tensor.matmul(
                    p2[:, 0:cols], cc("w2", rows=64, w=32), x1[:, 0:cols],
                    start=True, stop=True,
                )
                x2 = ropool.tile([32, 512], F32, tag="x2")
                nc.scalar.activation(
                    x2[:, 0:cols], p2[:, 0:cols], Relu, bias=cc("b2", rows=32)
                )
                p3 = psz.tile([128, 512], F32, tag="pz", name="ro_p3")
                nc.tensor.matmul(
                    p3[:, 0:cols], cc("w3", rows=32, w=128), x2[:, 0:cols],
                    start=True, stop=True,
                )
                x3 = ropool.tile([128, 512], F32, tag="x3")
                nc.scalar.activation(
                    x3[:, 0:cols], p3[:, 0:cols], Ident, bias=cc("b3")
                )
                for j in range(0, cols, 128):
                    b = (off + j) // 128
                    rows = LAST_ROWS if b == NB - 1 else 128
                    pt = pst.tile([128, 128], F32, tag="pt")
                    nc.tensor.transpose(pt[:], x3[:, j : j + 128], ident_ap)
                    ob = hbpool.tile([128, 128], F32, tag="hb")
                    nc.scalar.copy(ob[:], pt[:])
                    nc.sync.dma_start(
                        out_d[b * 128 : b * 128 + rows, :], ob[0:rows, :]
                    )

            # ---- setup: h0 = emb[labels] + pos_table[inv_perm] (bf16) ----
            ge = gopool.tile([128, NSLOT], BF16, tag="gown")
            hb0 = pp.tile([128, NSLOT], BF16, tag="hb0")
            e0 = ntiles * 8
            nc.gpsimd.dma_gather(
                ge[:, 0:NSLOT].rearrange("p (t e) -> p t e", e=D),
                emb_in[:, :],
                idx_t[:, e0 : e0 + NSLOT // 16],
                NSLOT, NSLOT, D, single_packet=False, queue_num=next_q(),
            )
            nc.gpsimd.dma_gather(
                hb0[:].rearrange("p (t e) -> p t e", e=D),
                pos_in[:, :],
                idx_t[:, e0 + NSLOT // 16 : e0 + 2 * (NSLOT // 16)],
                NSLOT, NSLOT, D, single_packet=False, queue_num=next_q(),
            )
            nc.vector.tensor_add(hb0[:], ge[:, 0:NSLOT], hb0[:])
            for b in range(NB):
                rows = LAST_ROWS if b == NB - 1 else 128
                nc.sync.dma_start(
                    hgb[b * 128 : b * 128 + rows, :],
                    hb0[0:rows, b * 128 : (b + 1) * 128],
                )
            for b in range(NB):
                ptb = pst.tile([128, 128], BF16, tag="pt")
                nc.tensor.transpose(
                    ptb[:], hb0[:, b * 128 : (b + 1) * 128], identb[:]
                )
                nc.scalar.copy(hT[:, b * 128 : (b + 1) * 128], ptb[:])
            ag1(0)
            ag2(0)

            # ---- GCN layers ----
            for l in range(int(os.environ.get("GCN_NLAYERS", NLAYERS))):
                par = l
                nlayers_run = int(os.environ.get("GCN_NLAYERS", NLAYERS))
                g_own = gopool.tile([128, own_tiles * 128], BF16, tag="gown")
                for ot in range(0, own_tiles, 32):
                    if SKIP_GATHER:
                        break
                    on = min(32, own_tiles - ot)
                    nc.gpsimd.dma_gather(
                        g_own[:, ot * 128 : (ot + on) * 128].rearrange(
                            "p (t e) -> p t e", e=D
                        ),
                        hgb[:, :],
                        idx_t[:, ot * 8 : (ot + on) * 8],
                        on * 128, on * 128, D, single_packet=False,
                        queue_num=next_q(),
                    )
                # phase 1: own+A aggregation per block (partials to SBUF);
                # phase 2 lags LAG chunks behind, adds the B contribution and
                # finishes the block (W matmul, residual, writeback). The lag
                # hides the previous layer's AG2 transfer.
                gA_t, gB_t = [None] * nchunks, [None] * nchunks

                def emit_A(k):
                    blocks, t0, TA, TB = chunks[k]
                    if TA == 0:
                        return
                    gA_t[k] = gapool.tile([128, tamax * 128], BF16, tag="gA", name=f"gA_l{l}_{k}")
                    qs = [0, TA // 4, TA // 2, (3 * TA) // 4, TA]
                    for h0, h1 in zip(qs[:-1], qs[1:]):
                        if h1 == h0 or SKIP_GATHER:
                            continue
                        nc.gpsimd.dma_gather(
                            gA_t[k][:, h0 * 128 : h1 * 128].rearrange(
                                "p (t e) -> p t e", e=D
                            ),
                            hgA[par][:, :],
                            idx_t[:, (t0 + h0) * 8 : (t0 + h1) * 8],
                            (h1 - h0) * 128, (h1 - h0) * 128, D,
                            single_packet=False, queue_num=next_q(),
                        )

                def emit_B(k):
                    blocks, t0, TA, TB = chunks[k]
                    if TB == 0:
                        return
                    gB_t[k] = gbpool.tile([128, tbmax * 128], BF16, tag="gB", name=f"gB_l{l}_{k}")
                    qs = [0, TB // 4, TB // 2, (3 * TB) // 4, TB]
                    for h0, h1 in zip(qs[:-1], qs[1:]):
                        if h1 == h0 or SKIP_GATHER:
                            continue
                        nc.gpsimd.dma_gather(
                            gB_t[k][:, h0 * 128 : h1 * 128].rearrange(
                                "p (t e) -> p t e", e=D
                            ),
                            hgB[par][:, :],
                            idx_t[:, (t0 + TA + h0) * 8 : (t0 + TA + h1) * 8],
                            (h1 - h0) * 128, (h1 - h0) * 128, D,
                            single_packet=False, queue_num=next_q(),
                        )

                oh1_t = [None] * nchunks
                oh2_t = [None] * nchunks

                def emit_oh1(k):
                    oh1_t[k] = oh1pool.tile(
                        [128, oh1max * 128], F8, tag="oh1", name=f"oh1_l{l}_{k}"
                    )
                    if SKIP_OH:
                        return
                    c0 = int(slab10[k]) * 128
                    cw = chunk_tiles1[k] * 128
                    nc.sync.dma_start(oh1_t[k][:, 0:cw], oh1_in[:, c0 : c0 + cw])

                def emit_oh2(k):
                    if chunk_tiles2[k] == 0:
                        return
                    oh2_t[k] = oh2pool.tile(
                        [128, oh2max * 128], F8, tag="oh2", name=f"oh2_l{l}_{k}"
                    )
                    if SKIP_OH:
                        return
                    c0 = int(slab20[k]) * 128
                    cw = chunk_tiles2[k] * 128
                    nc.sync.dma_start(oh2_t[k][:, 0:cw], oh2_in[:, c0 : c0 + cw])

                mTp_t = [None] * nchunks

                def phase1(k):
                    blocks, t0, TA, TB = chunks[k]
                    mTp_t[k] = mtppool.tile(
                        [128, 512], F32, tag="mTp", name=f"mTp_l{l}_{k}"
                    )
                    pos = 0
                    for j, b in enumerate(blocks):
                        tl = p1_tiles[b]
                        if not tl:
                            nc.vector.memset(
                                mTp_t[k][:, j * 128 : (j + 1) * 128], 0.0
                            )
                            continue
                        pm = psm.tile([128, 128], F32, tag="pm")
                        for i, t in enumerate(tl):
                            oh = oh1_t[k][:, pos * 128 : (pos + 1) * 128]
                            pos += 1
                            lhs = gA_t[k][:, (t - t0) * 128 : (t - t0 + 1) * 128]
                            nc.tensor.matmul(
                                pm[:], lhs, oh,
                                start=(i == 0),
                                stop=(i == len(tl) - 1),
                            )
                        nc.scalar.copy(mTp_t[k][:, j * 128 : (j + 1) * 128], pm[:])

                def phase2(k):
                    blocks, t0, TA, TB = chunks[k]
                    mT = mtpool.tile([128, 512], BF16, tag="mT")
                    pos = 0
                    for j, b in enumerate(blocks):
                        tl = p2_tiles[b]
                        jc = slice(j * 128, (j + 1) * 128)
                        if tl:
                            pm = psm.tile([128, 128], F32, tag="pm")
                            for i, t in enumerate(tl):
                                oh = oh2_t[k][:, pos * 128 : (pos + 1) * 128]
                                pos += 1
                                if t < own_tiles:
                                    lhs = g_own[:, t * 128 : (t + 1) * 128]
                                else:
                                    lhs = gB_t[k][
                                        :,
                                        (t - t0 - TA) * 128 : (t - t0 - TA + 1) * 128,
                                    ]
                                nc.tensor.matmul(
                                    pm[:], lhs, oh,
                                    start=(i == 0),
                                    stop=(i == len(tl) - 1),
                                )
                            c2 = c2pool.tile([128, 128], F32, tag="c2")
                            nc.scalar.copy(c2[:], pm[:])
                            nc.vector.tensor_add(
                                mT[:, jc], mTp_t[k][:, jc], c2[:]
                            )
                        else:
                            nc.scalar.copy(mT[:, jc], mTp_t[k][:, jc])
                    cols = len(blocks) * 128
                    pz = psz.tile([128, 512], F32, tag="pz")
                    nc.tensor.matmul(
                        pz[:, 0:cols],
                        w4b[:, l * 128 : (l + 1) * 128],
                        mT[:, 0:cols],
                        start=True, stop=True,
                    )
                    zr = zrpool.tile([128, 512], F32, tag="zr")
                    nc.scalar.activation(
                        zr[:, 0:cols], pz[:, 0:cols], Relu, bias=cc("b4", l)
                    )
                    c0 = blocks[0] * 128
                    nc.vector.tensor_add(
                        hT[:, c0 : c0 + cols], hT[:, c0 : c0 + cols], zr[:, 0:cols]
                    )
                    if l < nlayers_run - 1:
                        for b in blocks:
                            rows = LAST_ROWS if b == NB - 1 else 128
                            pt = pst.tile([128, 128], F32, tag="pt")
                            nc.tensor.transpose(
                                pt[:], hT[:, b * 128 : (b + 1) * 128], ident_ap
                            )
                            hb = hbpool.tile([128, 128], BF16, tag="hbw")
                            nc.scalar.copy(hb[:], pt[:])
                            nc.sync.dma_start(
                                hgb[b * 128 : b * 128 + rows, :], hb[0:rows, :]
                            )
                        if blocks[-1] == HBLK - 1:
                            ag1(l + 1)
                    else:
                        readout(c0, cols)

                for k in range(nchunks):
                    emit_A(k)
                emit_oh1(0)
                emit_oh1(1)
                oh2_next = [0]

                def ensure_oh2(upto):
                    while oh2_next[0] <= min(upto, nchunks - 1):
                        emit_oh2(oh2_next[0])
                        oh2_next[0] += 1

                for k in range(nchunks):
                    emit_B(k)
                    if k + 2 < nchunks:
                        emit_oh1(k + 2)
                    phase1(k)
                    k2 = k - LAG
                    if k2 >= 0:
                        ensure_oh2(k2 + 1)
                        phase2(k2)
                for k2 in range(max(0, nchunks - LAG), nchunks):
                    ensure_oh2(k2 + 1)
                    phase2(k2)
                if l < nlayers_run - 1:
                    ag2(l + 1)

    nc.compile()
    return nc


last_results = None


def kernel(labels, src, dst, perms, emb, Ws, bs, w1, b1, w2, b2, w3, b3):
    global last_results
    meta, per_core = _preprocess(labels, src, dst, perms)
    cst0, ccols = _build_cst(Ws, bs, w1, b1, w2, b2, w3, b3)
    key = (meta["ntiles"], meta["own_tiles"], meta["n1"], meta["n2"],
           os.environ.get("GCN_NLAYERS", ""), os.environ.get("GCN_LAG", ""),
           os.environ.get("GCN_SKIP_AG", ""),
           os.environ.get("GCN_SKIP_GATHER", ""),
           os.environ.get("GCN_SKIP_OH", ""),
           tuple(t for _, t, _, _ in meta["chunks"]))
    if key not in _cache:
        _cache[key] = _build_nc(
            meta, cst0.shape[1], ccols, per_core[0]["idxall"].shape[1]
        )
    nc = _cache[key]

    import ml_dtypes
    emb_np = np.asarray(emb, np.float32).astype(ml_dtypes.bfloat16)
    pos_np = _pos_table().astype(ml_dtypes.bfloat16)
    idb_np = np.eye(128, dtype=ml_dtypes.bfloat16)
    w4b_np = np.concatenate(
        [np.asarray(Ws[l], np.float32) for l in range(NLAYERS)], 1
    ).astype(ml_dtypes.bfloat16)
    in_maps = []
    for c in range(NCORES):
        oh1_c, oh2_c = _build_ohall(
            meta, per_core[c]["slot"], per_core[c]["dl"], per_core[c]["se"]
        )
        in_maps.append(
            dict(idxall=per_core[c]["idxall"], cst=cst0, emb=emb_np, pos=pos_np,
                 w4b=w4b_np, identb=idb_np, ohall1=oh1_c, ohall2=oh2_c)
        )
    res = run_bass_kernel_spmd(nc, in_maps, core_ids=list(range(NCORES)))
    last_results = res
    cat = np.concatenate([res.results[c]["out"] for c in range(NCORES)], axis=0)
    out = np.empty_like(cat)
    out[meta["old_of"]] = cat
    return out

